# revision 9
# baseline (speedup 1.0000x reference)
"""Causal self-attention (B=4, T=2048, E=512, H=8) on 8 TRN2 NeuronCores.

Sharding: core c -> (batch b = c//2, head-group hg = c%2, 4 heads each).
Host sums the two partial projection outputs per batch and adds the fused
output bias (bp + bv @ Wp; softmax weights sum to 1 so the v-bias commutes
out of attention).

Design (v3) — all f16, restructured around PE array utilization:
- qkv proj per token group (as v2): q/k feature-major (qkT), v token-major.
- Scores as S^T per (head, key-block): K=64 contraction; head pairs at
  partition rows 0-63/64-127 -> the two matmuls run CONCURRENTLY on
  disjoint PE row-groups.
- exp split ScalarE (exact) / DVE (Schraudolph f16 bit-trick). Diagonal
  staircase masking is FUSED into the DVE exp via scalar_tensor_tensor:
  out_i16 = pS*A16 + maskB, where masked lanes get -1e5 -> i16 saturates to
  0x8000 = f16 -0.0 (verified on HW).
- PV in "y-form": y[q,d] += p_block[:,q].T @ v_block -> M=128 output
  partitions (full PE width; 2x fewer cycles than the M=65 v-stationary
  form). Denominator via an extra N=1 matmul on the same stationary p
  (rhs = ones column) into a separate PSUM bank.
- Normalize: DVE reciprocal of den [128,16], then ScalarE Copy with
  per-partition scale AP fuses the 1/den multiply into the PSUM->SBUF move.
  No cross-partition broadcasts at all.
- y -> yT via PE transpose (f16 PSUM out) + DVE copy; output projection
  K=128 x2 accumulating matmuls; z DMAed PSUM->DRAM in f32 (bias on host).
"""

from contextlib import ExitStack
from itertools import chain as _chain

import numpy as np

import concourse.bass as bass
import concourse.mybir as mybir
import concourse.tile as tile
from concourse import bacc
from concourse.bass import ts
from concourse.bass_utils import run_bass_kernel_spmd

f32 = mybir.dt.float32
f16 = mybir.dt.float16
i16 = mybir.dt.int16
FA = mybir.ActivationFunctionType
MUL = mybir.AluOpType.mult
ADD = mybir.AluOpType.add

B, T, E = 4, 2048, 512
H, D = 8, 64
HPC = 4              # heads per core
EC = HPC * D         # 256
P = 128
NCORES = 8
TQ = T // P          # 16 token chunks
NQG = T // 512       # 4 query groups
EO = E // P          # 4 contraction subtiles
SCALE = 1.0 / np.sqrt(D)

# Schraudolph fp16 fast-exp constants (round-half-even on DVE f32->i16)
A16 = float(2.0**10 / np.log(2.0))
B16 = 15360.0 - 59.0
MASKNEG = -1.0e5     # (S*A16 + MASKNEG) saturates i16 -> 0x8000 = f16 -0.0

# act8: of every 8 exp tiles, this many go to ScalarE (rest DVE)
CFG = {"act8": 4, "pS_bufs": 3, "expS_bufs": 6, "xT_bufs": 2}


def _emit(tc, ctx, aps, reps=1):
    nc = tc.nc
    z = aps["z"]

    cst = ctx.enter_context(tc.tile_pool(name="cst", bufs=1))
    wqk_sb = cst.tile([P, EO, 2 * EC], f16)
    for eo in range(EO):
        nc.sync.dma_start(wqk_sb[:, eo, :], aps["wqk"][:, eo, :])
    bqk_sb = cst.tile([P, 4], f32)
    nc.sync.dma_start(bqk_sb, aps["bqk"])
    wv_sb = cst.tile([P, EO, EC], f16)
    nc.sync.dma_start(wv_sb, aps["wv"])
    wp_sb = cst.tile([P, 2, E], f16)
    nc.sync.dma_start(wp_sb, aps["wp"])
    ident = cst.tile([P, P], f16)
    nc.sync.dma_start(ident, aps["ident"])
    maskB = cst.tile([P, P], f32)
    nc.sync.dma_start(maskB, aps["maskB"])
    ones1 = cst.tile([P, 1], f16)
    nc.vector.memset(ones1, 1.0)

    big = ctx.enter_context(tc.tile_pool(name="big", bufs=1))
    qkT = big.tile([P, 4, T], f16)           # sub 0-1: q^T, 2-3: k^T
    v4 = big.tile([P, TQ, HPC, D], f16)      # v token-major per (block, head)
    yT = big.tile([P, 2, T], f16)            # [0:64]+[64:128] v-dims per e-tile

    xTp = ctx.enter_context(tc.tile_pool(name="xTp", bufs=CFG["xT_bufs"]))
    expSp = ctx.enter_context(tc.tile_pool(name="expSp", bufs=CFG["expS_bufs"]))
    ynp = ctx.enter_context(tc.tile_pool(name="ynp", bufs=3))
    rcpp = ctx.enter_context(tc.tile_pool(name="rcpp", bufs=2))
    zp = ctx.enter_context(tc.tile_pool(name="zp", bufs=3))

    pS = ctx.enter_context(tc.tile_pool(name="pS", bufs=CFG["pS_bufs"], space="PSUM"))
    pY = ctx.enter_context(tc.tile_pool(name="pY", bufs=1, space="PSUM"))
    pD = ctx.enter_context(tc.tile_pool(name="pD", bufs=1, space="PSUM"))
    pG = ctx.enter_context(tc.tile_pool(name="pG", bufs=2, space="PSUM"))

    exp_ctr = [0]

    def emit_exp(pSt, kb, is_diag, w, expS):
        """exp of score block [128, w] (left-packed) into expS[:, kb, 0:w].
        Diag blocks' first 128 cols get the fused staircase mask on DVE;
        the rest is split ScalarE/DVE by CFG ratio."""
        lo = 0
        if is_diag:
            nc.vector.scalar_tensor_tensor(
                expS[:, kb, 0:128].bitcast(i16), pSt[:, 0:128], A16, maskB,
                MUL, ADD,
            )
            lo = 128
            if w <= lo:
                return
        use_act = (exp_ctr[0] % 8) < CFG["act8"]
        exp_ctr[0] += 1
        if use_act:
            nc.scalar.activation(expS[:, kb, lo:w], pSt[:, lo:w], FA.Exp)
        else:
            nc.vector.tensor_scalar(
                expS[:, kb, lo:w].bitcast(i16), pSt[:, lo:w], A16, B16,
                MUL, ADD,
            )

    def phase1_chunks(xT, tg):
        """qkv projection for token group tg, yield per chunk."""
        for jc in range(4):
            pq = pG.tile([P, 512], f32, tag="g", name=f"pq_{tg}_{jc}")
            for eo in range(EO):
                nc.tensor.matmul(
                    pq,
                    lhsT=wqk_sb[:, eo, ts(jc, P)],
                    rhs=xT[:, eo, ts(tg, 512)],
                    start=(eo == 0),
                    stop=(eo == EO - 1),
                )
            nc.scalar.activation(
                qkT[:, jc, ts(tg, 512)], pq, FA.Identity, bias=bqk_sb[:, jc : jc + 1]
            )
            yield
        for j in range(4):
            tq = 4 * tg + j
            pv = pG.tile([P, 512], f32, tag="g", name=f"pv_{tq}")
            for eo in range(EO):
                nc.tensor.matmul(
                    pv[:, :EC],
                    lhsT=xT[:, eo, ts(tq, P)],
                    rhs=wv_sb[:, eo, :],
                    start=(eo == 0),
                    stop=(eo == EO - 1),
                )
            nc.scalar.activation(
                v4[:, tq, :, :],
                pv[:, :EC].rearrange("p (h c) -> p h c", c=64),
                FA.Copy,
            )
            yield

    def load_xT():
        xT = xTp.tile([P, EO, T], f16, tag="x", name="xT_sb")
        for eo in range(0, EO, 2):
            for th in range(4):
                nc.sync.dma_start(
                    xT[:, eo : eo + 2, ts(th, T // 4)],
                    aps["xT"][:, eo : eo + 2, ts(th, T // 4)],
                )
        return xT

    def scores_chunks(qg, expS4):
        """S^T blocks + exp for all 4 heads of query group qg.

        Per (head, kb): one [128, w] matmul (w = 512 - 128*jj, left-packed
        diag trim), head pairs on disjoint PE row groups run concurrently.
        expS4 = 4 per-head expS tiles [P, 16, 512] f16."""
        nb = 4 * qg + 4
        for kb in range(nb):
            jj = max(kb - 4 * qg, 0)
            w = 512 - 128 * jj
            is_diag = kb >= 4 * qg
            for pair in range(2):
                q_sub, k_sub = pair, 2 + pair
                tiles = []
                for j in range(2):
                    h = 2 * pair + j
                    hp = j * 64
                    pSt = pS.tile([P, 512], f32, tag="s", name=f"pS_{qg}_{h}_{kb}")
                    tiles.append(pSt)
                    nc.tensor.matmul(
                        pSt[:, 0:w],
                        lhsT=qkT[hp : hp + 64, k_sub, ts(kb, P)],
                        rhs=qkT[
                            hp : hp + 64, q_sub,
                            qg * 512 + 128 * jj : (qg + 1) * 512,
                        ],
                        start=True,
                        stop=True,
                    )
                for j in range(2):
                    h = 2 * pair + j
                    emit_exp(tiles[j], kb, is_diag, w, expS4[h])
                yield

    def pv_y(qg, expS4, sgen):
        """y-form PV + den for query group qg.

        y[qchunk c][h] [128, 64] psum += expS[:, kb, chunk].T @ v4[:, kb, h]
        den[4c+h] [128, 1] psum    += expS[:, kb, chunk].T @ ones
        Pulls one scores chunk from sgen per (kb, h) to interleave."""
        nb = 4 * qg + 4
        yps = pY.tile([P, 4, HPC, D], f32, tag="y", name=f"yps_{qg}")
        dps = pD.tile([P, 16], f32, tag="d", name=f"dps_{qg}")
        # PSUM start=True lazily zeroes the WHOLE 2KB bank (ZERO_REGION), so
        # only the first matmul touching each bank may carry it: yps spans 2
        # banks (chunks 0-1 / 2-3), dps one. Later regions' first writes land
        # on pending-zero bytes and overwrite; accumulation then proceeds.
        for kb in range(nb):
            jj = max(kb - 4 * qg, 0)
            for h in range(HPC):
                for c in range(jj, 4):
                    lt = expS4[h][:, kb, 128 * (c - jj) : 128 * (c - jj) + 128]
                    nc.tensor.matmul(
                        yps[:, c, h, :],
                        lhsT=lt,
                        rhs=v4[:, kb, h, :],
                        start=(kb == 0 and h == 0 and c in (0, 2)),
                        stop=(kb == 4 * qg + c),
                        skip_group_check=True,
                    )
                    nc.tensor.matmul(
                        dps[:, 4 * c + h : 4 * c + h + 1],
                        lhsT=lt,
                        rhs=ones1,
                        start=(kb == 0 and h == 0 and c == 0),
                        stop=(kb == 4 * qg + c),
                        skip_group_check=True,
                    )
                _pull(sgen)
        return yps, dps

    def normalize(qg, yps, dps, sgen):
        """reciprocal (DVE) -> per-partition-scaled copies (ScalarE) ->
        transposes (PE) -> yT copies (DVE)."""
        rcp = rcpp.tile([P, 16], f32, tag="r", name=f"rcp_{qg}")
        nc.vector.reciprocal_approx_fast(rcp, dps)
        for c in range(4):
            yn = ynp.tile([P, HPC, D], f16, tag="n", name=f"yn_{qg}_{c}")
            for h in range(HPC):
                nc.scalar.activation(
                    yn[:, h, :], yps[:, c, h, :], FA.Copy,
                    scale=rcp[:, 4 * c + h : 4 * c + h + 1],
                )
            tpt = pG.tile([P, 2, P], f16, tag="g", name=f"tpt_{qg}_{c}")
            for e in range(2):
                nc.tensor.matmul(
                    tpt[:, e, :],
                    lhsT=yn[:, 2 * e : 2 * e + 2, :],
                    rhs=ident,
                    start=True,
                    stop=True,
                    is_transpose=True,
                )
            nc.vector.tensor_copy(
                yT[:, :, qg * 512 + 128 * c : qg * 512 + 128 * (c + 1)], tpt
            )
            _pull(sgen)

    def proj_z(qg, sgen):
        """output projection for the 4 token chunks of query group qg;
        PSUM -> SBUF f16 copy on DVE, then DMA (bias added on host)."""
        for tq in range(4 * qg, 4 * qg + 4):
            pz = pG.tile([P, 512], f32, tag="g", name=f"pz_{tq}")
            for e in range(2):
                nc.tensor.matmul(
                    pz,
                    lhsT=yT[:, e, ts(tq, P)],
                    rhs=wp_sb[:, e, :],
                    start=(e == 0),
                    stop=(e == 1),
                )
            zt = zp.tile([P, E], f16, tag="z", name=f"zt_{tq}")
            nc.vector.tensor_copy(zt, pz)
            nc.sync.dma_start(z[ts(tq, P), :], zt)
            _pull(sgen)

    def _pull(gen):
        if gen is not None:
            try:
                next(gen)
            except StopIteration:
                pass

    def _drain(gen):
        if gen is not None:
            for _ in gen:
                pass

    def phase1(xT, tg):
        _drain(phase1_chunks(xT, tg))

    def new_exp4(qg):
        return [
            expSp.tile([P, TQ, 512], f16, tag="e", name=f"exp{j}_{qg}")
            for j in range(4)
        ]

    xT = load_xT()
    phase1(xT, 0)
    exp4 = new_exp4(0)
    _drain(scores_chunks(0, exp4))
    for r in range(reps):
        last = r == reps - 1
        for qg in range(NQG):
            # dense PE work for qg, interleaved with scores+exp of qg+1
            if qg < NQG - 1:
                phase1(xT, qg + 1)
                exp4n = new_exp4(qg + 1)
                sgen = scores_chunks(qg + 1, exp4n)
            elif not last:
                xTn = load_xT()
                exp4n = new_exp4(0)
                sgen = _chain(phase1_chunks(xTn, 0), scores_chunks(0, exp4n))
            else:
                xTn, exp4n, sgen = None, None, None
            yps, dps = pv_y(qg, exp4, sgen)
            normalize(qg, yps, dps, sgen)
            if qg > 0:
                proj_z(qg - 1, sgen)
            _drain(sgen)
            exp4 = exp4n
            if qg == NQG - 1 and not last:
                xT = xTn
        proj_z(NQG - 1, None)


def build(reps=1):
    nc = bacc.Bacc("TRN2", target_bir_lowering=False, debug=False)
    aps = {
        "xT": nc.dram_tensor("xT", [P, EO, T], f16, kind="ExternalInput").ap(),
        "wqk": nc.dram_tensor("wqk", [P, EO, 2 * EC], f16, kind="ExternalInput").ap(),
        "bqk": nc.dram_tensor("bqk", [P, 4], f32, kind="ExternalInput").ap(),
        "wv": nc.dram_tensor("wv", [P, EO, EC], f16, kind="ExternalInput").ap(),
        "wp": nc.dram_tensor("wp", [P, 2, E], f16, kind="ExternalInput").ap(),
        "ident": nc.dram_tensor("ident", [P, P], f16, kind="ExternalInput").ap(),
        "maskB": nc.dram_tensor("maskB", [P, P], f32, kind="ExternalInput").ap(),
        "z": nc.dram_tensor("z", [T, E], f16, kind="ExternalOutput").ap(),
    }
    with tile.TileContext(nc) as tc, ExitStack() as ctx:
        _emit(tc, ctx, aps, reps=reps)
    nc.compile()
    return nc


def make_in_maps(x, c_attn_w, c_attn_b, c_proj_w, c_proj_b):
    x = np.asarray(x, np.float32)
    W = np.asarray(c_attn_w, np.float32)
    bW = np.asarray(c_attn_b, np.float32)

    ident = np.eye(P, dtype=np.float16)
    # maskB[p, c] = B16 if query col c >= key row p else -1e5 (staircase)
    pp = np.arange(P)[:, None]
    cc = np.arange(P)[None, :]
    maskB = np.where(cc >= pp, B16, MASKNEG).astype(np.float32)
    in_maps = []
    for c in range(NCORES):
        b, hg = c // 2, c % 2
        qs = slice(hg * EC, (hg + 1) * EC)
        ks = slice(E + hg * EC, E + (hg + 1) * EC)
        vs = slice(2 * E + hg * EC, 2 * E + (hg + 1) * EC)
        wqk = np.concatenate([W[:, qs] * SCALE, W[:, ks]], axis=1)  # [512, 512]
        bqk = np.concatenate([bW[qs] * SCALE, bW[ks]])              # [512]
        xT = np.ascontiguousarray(
            x[b].T.reshape(EO, P, T).transpose(1, 0, 2)
        ).astype(np.float16)
        Wp_core = np.asarray(c_proj_w, np.float32)[hg * EC : (hg + 1) * EC, :]
        in_maps.append({
            "xT": xT,
            "wqk": np.ascontiguousarray(
                wqk.reshape(EO, P, 2 * EC).transpose(1, 0, 2)
            ).astype(np.float16),
            "bqk": np.ascontiguousarray(bqk.reshape(4, P).T),
            "wv": np.ascontiguousarray(
                W[:, vs].reshape(EO, P, EC).transpose(1, 0, 2)
            ).astype(np.float16),
            "wp": np.ascontiguousarray(
                Wp_core.reshape(2, P, E).transpose(1, 0, 2)
            ).astype(np.float16),
            "ident": ident,
            "maskB": maskB,
        })
    return in_maps


_NC_CACHE = {}


def kernel(x, c_attn_w, c_attn_b, c_proj_w, c_proj_b):
    if "nc" not in _NC_CACHE:
        _NC_CACHE["nc"] = build()
    nc = _NC_CACHE["nc"]
    in_maps = make_in_maps(x, c_attn_w, c_attn_b, c_proj_w, c_proj_b)
    res = run_bass_kernel_spmd(nc, in_maps, core_ids=list(range(NCORES)))
    bW = np.asarray(c_attn_b, np.float32)
    Wp = np.asarray(c_proj_w, np.float32)
    bias = np.asarray(c_proj_b, np.float32) + bW[2 * E :] @ Wp  # [512]
    out = np.empty((B, T, E), np.float32)
    for b in range(B):
        out[b] = (
            res.results[2 * b]["z"].astype(np.float32)
            + res.results[2 * b + 1]["z"].astype(np.float32)
            + bias[None, :]
        )
    return out


# revision 15
# speedup vs baseline: 1.1717x; 1.1717x over previous
"""Causal self-attention (B=4, T=2048, E=512, H=8) on 8 TRN2 NeuronCores.

Sharding: core c -> (batch b = c//2, head-group hg = c%2, 4 heads each).
Host sums the two partial projection outputs per batch and adds the fused
output bias (bp + bv @ Wp; softmax weights sum to 1 so the v-bias commutes
out of attention).

Design (v3) — all f16, restructured around PE array utilization:
- qkv proj per token group (as v2): q/k feature-major (qkT), v token-major.
- Scores as S^T per (head, key-block): K=64 contraction; head pairs at
  partition rows 0-63/64-127 -> the two matmuls run CONCURRENTLY on
  disjoint PE row-groups.
- exp split ScalarE (exact) / DVE (Schraudolph f16 bit-trick). Diagonal
  staircase masking is FUSED into the DVE exp via scalar_tensor_tensor:
  out_i16 = pS*A16 + maskB, where masked lanes get -1e5 -> i16 saturates to
  0x8000 = f16 -0.0 (verified on HW).
- PV in "y-form": y[q,d] += p_block[:,q].T @ v_block -> M=128 output
  partitions (full PE width; 2x fewer cycles than the M=65 v-stationary
  form). Denominator via an extra N=1 matmul on the same stationary p
  (rhs = ones column) into a separate PSUM bank.
- Normalize: DVE reciprocal of den [128,16], then ScalarE Copy with
  per-partition scale AP fuses the 1/den multiply into the PSUM->SBUF move.
  No cross-partition broadcasts at all.
- y -> yT via PE transpose (f16 PSUM out) + DVE copy; output projection
  K=128 x2 accumulating matmuls; z DMAed PSUM->DRAM in f32 (bias on host).
"""

from contextlib import ExitStack
from itertools import chain as _chain

import numpy as np

import concourse.bass as bass
import concourse.mybir as mybir
import concourse.tile as tile
from concourse import bacc
from concourse.bass import ts
from concourse.bass_utils import run_bass_kernel_spmd

f32 = mybir.dt.float32
f16 = mybir.dt.float16
i16 = mybir.dt.int16
FA = mybir.ActivationFunctionType
MUL = mybir.AluOpType.mult
ADD = mybir.AluOpType.add

B, T, E = 4, 2048, 512
H, D = 8, 64
HPC = 4              # heads per core
EC = HPC * D         # 256
P = 128
NCORES = 8
TQ = T // P          # 16 token chunks
NQG = T // 512       # 4 query groups
EO = E // P          # 4 contraction subtiles
SCALE = 1.0 / np.sqrt(D)

# Schraudolph fp16 fast-exp constants (round-half-even on DVE f32->i16)
A16 = float(2.0**10 / np.log(2.0))
B16 = 15360.0 - 59.0
MASKNEG = -1.0e5     # (S*A16 + MASKNEG) saturates i16 -> 0x8000 = f16 -0.0

# act8: of every 8 exp tiles, this many go to ScalarE (rest DVE)
CFG = {"act8": 4, "pS_bufs": 2, "expS_bufs": 6, "xT_bufs": 2}


def _emit(tc, ctx, aps, reps=1):
    nc = tc.nc
    z = aps["z"]

    cst = ctx.enter_context(tc.tile_pool(name="cst", bufs=1))
    wqk_sb = cst.tile([P, EO, 2 * EC], f16)
    for eo in range(EO):
        nc.sync.dma_start(wqk_sb[:, eo, :], aps["wqk"][:, eo, :])
    bqk_sb = cst.tile([P, 4], f32)
    nc.sync.dma_start(bqk_sb, aps["bqk"])
    wv_sb = cst.tile([P, EO, EC], f16)
    nc.sync.dma_start(wv_sb, aps["wv"])
    wp_sb = cst.tile([P, 2, E], f16)
    nc.sync.dma_start(wp_sb, aps["wp"])
    ident = cst.tile([P, P], f16)
    nc.sync.dma_start(ident, aps["ident"])
    maskB = cst.tile([P, 1, P], f32)
    nc.sync.dma_start(maskB, aps["maskB"])

    big = ctx.enter_context(tc.tile_pool(name="big", bufs=1))
    qkT = big.tile([P, 4, T], f16)           # sub 0-1: q^T, 2-3: k^T
    v4 = big.tile([P, TQ, HPC, 65], f16)     # v token-major + ones col 64
    yT = big.tile([P, 2, T], f16)            # [0:64]+[64:128] v-dims per e-tile

    xTp = ctx.enter_context(tc.tile_pool(name="xTp", bufs=CFG["xT_bufs"]))
    expSp = ctx.enter_context(tc.tile_pool(name="expSp", bufs=CFG["expS_bufs"]))
    ynp = ctx.enter_context(tc.tile_pool(name="ynp", bufs=3))
    rcpp = ctx.enter_context(tc.tile_pool(name="rcpp", bufs=2))
    zp = ctx.enter_context(tc.tile_pool(name="zp", bufs=3))

    pS = ctx.enter_context(tc.tile_pool(name="pS", bufs=CFG["pS_bufs"], space="PSUM"))
    pY = ctx.enter_context(tc.tile_pool(name="pY", bufs=2, space="PSUM"))
    pG = ctx.enter_context(tc.tile_pool(name="pG", bufs=2, space="PSUM"))

    exp_ctr = [0]

    def emit_exp(pSt, pr, is_diag, wB, expS):
        """exp of a [128, 1024] score pair-tile (blocks 2pr, 2pr+1) into
        expS[:, pr, :]. Full pairs: one instr, engine by CFG ratio. Diag
        pairs: one plain exp over [128 : 512+wB], then one fused staircase
        mask-exp (DVE) overwriting cols [0:128] and [512:640] via a strided
        AP. Junk cols between a short block A and col 512 are exp'd but
        never read."""
        use_act = (exp_ctr[0] % 8) < CFG["act8"]
        exp_ctr[0] += 1
        if not is_diag:
            if use_act:
                nc.scalar.activation(expS[:, pr, :], pSt, FA.Exp)
            else:
                nc.vector.tensor_scalar(
                    expS[:, pr, :].bitcast(i16), pSt, A16, B16, MUL, ADD
                )
            return
        hi = 512 + wB
        if use_act:
            nc.scalar.activation(expS[:, pr, 128:hi], pSt[:, 128:hi], FA.Exp)
        else:
            nc.vector.tensor_scalar(
                expS[:, pr, 128:hi].bitcast(i16), pSt[:, 128:hi], A16, B16,
                MUL, ADD,
            )
        ev = expS[:, pr, :].rearrange("p (a b) -> p a b", b=512)
        pv_ = pSt.rearrange("p (a b) -> p a b", b=512)
        nc.vector.scalar_tensor_tensor(
            ev[:, :, 0:128].bitcast(i16), pv_[:, :, 0:128], A16,
            maskB[:, 0:1, :].to_broadcast((P, 2, 128)), MUL, ADD,
        )

    def phase1_chunks(xT, tg):
        """qkv projection for token group tg, yield per chunk."""
        for jc in range(4):
            pq = pG.tile([P, 512], f32, tag="g", name=f"pq_{tg}_{jc}")
            for eo in range(EO):
                nc.tensor.matmul(
                    pq,
                    lhsT=wqk_sb[:, eo, ts(jc, P)],
                    rhs=xT[:, eo, ts(tg, 512)],
                    start=(eo == 0),
                    stop=(eo == EO - 1),
                )
            nc.scalar.activation(
                qkT[:, jc, ts(tg, 512)], pq, FA.Identity, bias=bqk_sb[:, jc : jc + 1]
            )
            yield
        for j in range(4):
            tq = 4 * tg + j
            pv = pG.tile([P, 512], f32, tag="g", name=f"pv_{tq}")
            for eo in range(EO):
                nc.tensor.matmul(
                    pv[:, :EC],
                    lhsT=xT[:, eo, ts(tq, P)],
                    rhs=wv_sb[:, eo, :],
                    start=(eo == 0),
                    stop=(eo == EO - 1),
                )
            nc.scalar.activation(
                v4[:, tq, :, 0:64],
                pv[:, :EC].rearrange("p (h c) -> p h c", c=64),
                FA.Copy,
            )
            yield

    def load_xT():
        xT = xTp.tile([P, EO, T], f16, tag="x", name="xT_sb")
        for eo in range(0, EO, 2):
            for th in range(4):
                nc.sync.dma_start(
                    xT[:, eo : eo + 2, ts(th, T // 4)],
                    aps["xT"][:, eo : eo + 2, ts(th, T // 4)],
                )
        return xT

    def scores_chunks(qg, expS4):
        """S^T block-pairs + exp for all 4 heads of query group qg.

        Per (head, pair pr): one [128, 1024] psum tile holding blocks
        (2pr, 2pr+1), each a left-packed [128, w] matmul; head pairs on
        disjoint PE row groups run concurrently.
        expS4 = 4 per-head expS tiles [P, 8, 1024] f16."""
        npr = 2 * qg + 2
        for pr in range(npr):
            is_diag = 2 * pr >= 4 * qg
            wB = 512 - 128 * max(2 * pr + 1 - 4 * qg, 0)
            for pair in range(2):
                q_sub, k_sub = pair, 2 + pair
                tiles = []
                for j in range(2):
                    h = 2 * pair + j
                    hp = j * 64
                    pSt = pS.tile([P, 1024], f32, tag="s", name=f"pS_{qg}_{h}_{pr}")
                    tiles.append(pSt)
                    for kk in range(2):
                        kb = 2 * pr + kk
                        jj = max(kb - 4 * qg, 0)
                        w = 512 - 128 * jj
                        nc.tensor.matmul(
                            pSt[:, 512 * kk : 512 * kk + w],
                            lhsT=qkT[hp : hp + 64, k_sub, ts(kb, P)],
                            rhs=qkT[
                                hp : hp + 64, q_sub,
                                qg * 512 + 128 * jj : (qg + 1) * 512,
                            ],
                            start=True,
                            stop=True,
                        )
                for j in range(2):
                    h = 2 * pair + j
                    emit_exp(tiles[j], pr, is_diag, wB, expS4[h])
                yield

    def pv_y(qg, expS4, sgen):
        """y-form PV + den for query group qg, one query chunk at a time.

        Per chunk c: ypc [128, 4h, 65] psum (one bank); for each key block
        kb <= 4qg+c and head h one matmul with rhs = [v | ones] [128, 65] —
        col 64 accumulates the softmax denominator. Then normalize+transpose
        for c while chunk c+1 accumulates (pY double-buffered).

        PSUM start=True lazily zeroes the whole 2KB bank, so only the first
        matmul of the chunk carries it; other heads' first writes land on
        pending-zero bytes and overwrite, then accumulate."""
        for c in range(4):
            ypc = pY.tile([P, HPC, 65], f32, tag="y", name=f"yps_{qg}_{c}")
            last_kb = 4 * qg + c
            for kb in range(last_kb + 1):
                jj = max(kb - 4 * qg, 0)
                off = 512 * (kb % 2) + 128 * (c - jj)
                for h in range(HPC):
                    nc.tensor.matmul(
                        ypc[:, h, :],
                        lhsT=expS4[h][:, kb // 2, off : off + 128],
                        rhs=v4[:, kb, h, :],
                        start=(kb == 0 and h == 0),
                        stop=(kb == last_kb),
                        skip_group_check=True,
                    )
                _pull(sgen)
            normalize_c(qg, c, ypc, sgen)

    def normalize_c(qg, c, ypc, sgen):
        """reciprocal of den col (DVE) -> batched broadcast-multiply (DVE)
        -> transposes (PE) -> yT copy (ScalarE)."""
        rcp = rcpp.tile([P, 4, 1], f32, tag="r", name=f"rcp_{qg}_{c}")
        nc.vector.reciprocal_approx_fast(rcp, ypc[:, :, 64:65])
        yn = ynp.tile([P, HPC, D], f16, tag="n", name=f"yn_{qg}_{c}")
        nc.vector.tensor_tensor(
            yn, ypc[:, :, 0:64], rcp.to_broadcast((P, HPC, D)), MUL
        )
        tpt = pG.tile([P, 2, P], f16, tag="g", name=f"tpt_{qg}_{c}")
        for e in range(2):
            nc.tensor.matmul(
                tpt[:, e, :],
                lhsT=yn[:, 2 * e : 2 * e + 2, :],
                rhs=ident,
                start=True,
                stop=True,
                is_transpose=True,
            )
        nc.scalar.activation(
            yT[:, :, qg * 512 + 128 * c : qg * 512 + 128 * (c + 1)], tpt,
            FA.Copy,
        )
        _pull(sgen)

    def proj_z(qg, sgen):
        """output projection for the 4 token chunks of query group qg;
        PSUM -> SBUF f16 copy on DVE, then DMA (bias added on host)."""
        for tq in range(4 * qg, 4 * qg + 4):
            pz = pG.tile([P, 512], f32, tag="g", name=f"pz_{tq}")
            for e in range(2):
                nc.tensor.matmul(
                    pz,
                    lhsT=yT[:, e, ts(tq, P)],
                    rhs=wp_sb[:, e, :],
                    start=(e == 0),
                    stop=(e == 1),
                )
            zt = zp.tile([P, E], f16, tag="z", name=f"zt_{tq}")
            nc.vector.tensor_copy(zt, pz)
            nc.sync.dma_start(z[ts(tq, P), :], zt)
            _pull(sgen)

    def _pull(gen):
        if gen is not None:
            try:
                next(gen)
            except StopIteration:
                pass

    def _drain(gen):
        if gen is not None:
            for _ in gen:
                pass

    def phase1(xT, tg):
        _drain(phase1_chunks(xT, tg))

    def new_exp4(qg):
        return [
            expSp.tile([P, 8, 1024], f16, tag="e", name=f"exp{j}_{qg}")
            for j in range(4)
        ]

    nc.vector.memset(v4[:, :, :, 64], 1.0)
    xT = load_xT()
    phase1(xT, 0)
    exp4 = new_exp4(0)
    _drain(scores_chunks(0, exp4))
    for r in range(reps):
        last = r == reps - 1
        for qg in range(NQG):
            # dense PE work for qg, interleaved with scores+exp of qg+1
            if qg < NQG - 1:
                phase1(xT, qg + 1)
                exp4n = new_exp4(qg + 1)
                sgen = scores_chunks(qg + 1, exp4n)
            elif not last:
                xTn = load_xT()
                exp4n = new_exp4(0)
                sgen = _chain(phase1_chunks(xTn, 0), scores_chunks(0, exp4n))
            else:
                xTn, exp4n, sgen = None, None, None
            pv_y(qg, exp4, sgen)
            if qg > 0:
                proj_z(qg - 1, sgen)
            _drain(sgen)
            exp4 = exp4n
            if qg == NQG - 1 and not last:
                xT = xTn
        proj_z(NQG - 1, None)


def build(reps=1):
    nc = bacc.Bacc("TRN2", target_bir_lowering=False, debug=False)
    aps = {
        "xT": nc.dram_tensor("xT", [P, EO, T], f16, kind="ExternalInput").ap(),
        "wqk": nc.dram_tensor("wqk", [P, EO, 2 * EC], f16, kind="ExternalInput").ap(),
        "bqk": nc.dram_tensor("bqk", [P, 4], f32, kind="ExternalInput").ap(),
        "wv": nc.dram_tensor("wv", [P, EO, EC], f16, kind="ExternalInput").ap(),
        "wp": nc.dram_tensor("wp", [P, 2, E], f16, kind="ExternalInput").ap(),
        "ident": nc.dram_tensor("ident", [P, P], f16, kind="ExternalInput").ap(),
        "maskB": nc.dram_tensor("maskB", [P, P], f32, kind="ExternalInput").ap(),
        "z": nc.dram_tensor("z", [T, E], f16, kind="ExternalOutput").ap(),
    }
    with tile.TileContext(nc) as tc, ExitStack() as ctx:
        _emit(tc, ctx, aps, reps=reps)
    nc.compile()
    return nc


def make_in_maps(x, c_attn_w, c_attn_b, c_proj_w, c_proj_b):
    x = np.asarray(x, np.float32)
    W = np.asarray(c_attn_w, np.float32)
    bW = np.asarray(c_attn_b, np.float32)

    ident = np.eye(P, dtype=np.float16)
    # maskB[p, c] = B16 if query col c >= key row p else -1e5 (staircase)
    pp = np.arange(P)[:, None]
    cc = np.arange(P)[None, :]
    maskB = np.where(cc >= pp, B16, MASKNEG).astype(np.float32)
    in_maps = []
    for c in range(NCORES):
        b, hg = c // 2, c % 2
        qs = slice(hg * EC, (hg + 1) * EC)
        ks = slice(E + hg * EC, E + (hg + 1) * EC)
        vs = slice(2 * E + hg * EC, 2 * E + (hg + 1) * EC)
        wqk = np.concatenate([W[:, qs] * SCALE, W[:, ks]], axis=1)  # [512, 512]
        bqk = np.concatenate([bW[qs] * SCALE, bW[ks]])              # [512]
        xT = np.ascontiguousarray(
            x[b].T.reshape(EO, P, T).transpose(1, 0, 2)
        ).astype(np.float16)
        Wp_core = np.asarray(c_proj_w, np.float32)[hg * EC : (hg + 1) * EC, :]
        in_maps.append({
            "xT": xT,
            "wqk": np.ascontiguousarray(
                wqk.reshape(EO, P, 2 * EC).transpose(1, 0, 2)
            ).astype(np.float16),
            "bqk": np.ascontiguousarray(bqk.reshape(4, P).T),
            "wv": np.ascontiguousarray(
                W[:, vs].reshape(EO, P, EC).transpose(1, 0, 2)
            ).astype(np.float16),
            "wp": np.ascontiguousarray(
                Wp_core.reshape(2, P, E).transpose(1, 0, 2)
            ).astype(np.float16),
            "ident": ident,
            "maskB": maskB,
        })
    return in_maps


_NC_CACHE = {}


def kernel(x, c_attn_w, c_attn_b, c_proj_w, c_proj_b):
    if "nc" not in _NC_CACHE:
        _NC_CACHE["nc"] = build()
    nc = _NC_CACHE["nc"]
    in_maps = make_in_maps(x, c_attn_w, c_attn_b, c_proj_w, c_proj_b)
    res = run_bass_kernel_spmd(nc, in_maps, core_ids=list(range(NCORES)))
    bW = np.asarray(c_attn_b, np.float32)
    Wp = np.asarray(c_proj_w, np.float32)
    bias = np.asarray(c_proj_b, np.float32) + bW[2 * E :] @ Wp  # [512]
    out = np.empty((B, T, E), np.float32)
    for b in range(B):
        out[b] = (
            res.results[2 * b]["z"].astype(np.float32)
            + res.results[2 * b + 1]["z"].astype(np.float32)
            + bias[None, :]
        )
    return out


# revision 19
# speedup vs baseline: 1.1819x; 1.0087x over previous
"""Causal self-attention (B=4, T=2048, E=512, H=8) on 8 TRN2 NeuronCores.

Sharding: core c -> (batch b = c//2, head-group hg = c%2, 4 heads each).
Host sums the two partial projection outputs per batch and adds the fused
output bias (bp + bv @ Wp; softmax weights sum to 1 so the v-bias commutes
out of attention).

Design (v3) — all f16, restructured around PE array utilization:
- qkv proj per token group (as v2): q/k feature-major (qkT), v token-major.
- Scores as S^T per (head, key-block): K=64 contraction; head pairs at
  partition rows 0-63/64-127 -> the two matmuls run CONCURRENTLY on
  disjoint PE row-groups.
- exp split ScalarE (exact) / DVE (Schraudolph f16 bit-trick). Diagonal
  staircase masking is FUSED into the DVE exp via scalar_tensor_tensor:
  out_i16 = pS*A16 + maskB, where masked lanes get -1e5 -> i16 saturates to
  0x8000 = f16 -0.0 (verified on HW).
- PV in "y-form": y[q,d] += p_block[:,q].T @ v_block -> M=128 output
  partitions (full PE width; 2x fewer cycles than the M=65 v-stationary
  form). Denominator via an extra N=1 matmul on the same stationary p
  (rhs = ones column) into a separate PSUM bank.
- Normalize: DVE reciprocal of den [128,16], then ScalarE Copy with
  per-partition scale AP fuses the 1/den multiply into the PSUM->SBUF move.
  No cross-partition broadcasts at all.
- y -> yT via PE transpose (f16 PSUM out) + DVE copy; output projection
  K=128 x2 accumulating matmuls; z DMAed PSUM->DRAM in f32 (bias on host).
"""

from contextlib import ExitStack
from itertools import chain as _chain

import numpy as np

import concourse.bass as bass
import concourse.mybir as mybir
import concourse.tile as tile
from concourse import bacc
from concourse.bass import ts
from concourse.bass_utils import run_bass_kernel_spmd

f32 = mybir.dt.float32
f16 = mybir.dt.float16
i16 = mybir.dt.int16
FA = mybir.ActivationFunctionType
MUL = mybir.AluOpType.mult
ADD = mybir.AluOpType.add

B, T, E = 4, 2048, 512
H, D = 8, 64
HPC = 4              # heads per core
EC = HPC * D         # 256
P = 128
NCORES = 8
TQ = T // P          # 16 token chunks
NQG = T // 512       # 4 query groups
EO = E // P          # 4 contraction subtiles
SCALE = 1.0 / np.sqrt(D)

# Schraudolph fp16 fast-exp constants (round-half-even on DVE f32->i16)
A16 = float(2.0**10 / np.log(2.0))
B16 = 15360.0 - 59.0
MASKNEG = -1.0e5     # (S*A16 + MASKNEG) saturates i16 -> 0x8000 = f16 -0.0

# act8: of every 8 exp tiles, this many go to ScalarE (rest DVE)
CFG = {"act8": 4, "pS_bufs": 2, "expS_bufs": 6, "xT_bufs": 2}


def _emit(tc, ctx, aps, reps=1):
    nc = tc.nc
    z = aps["z"]

    cst = ctx.enter_context(tc.tile_pool(name="cst", bufs=1))
    wqk_sb = cst.tile([P, EO, 2 * EC], f16)
    for eo in range(EO):
        nc.sync.dma_start(wqk_sb[:, eo, :], aps["wqk"][:, eo, :])
    bqk_sb = cst.tile([P, 4], f32)
    nc.sync.dma_start(bqk_sb, aps["bqk"])
    wv_sb = cst.tile([P, EO, EC], f16)
    nc.sync.dma_start(wv_sb, aps["wv"])
    wp_sb = cst.tile([P, 2, E], f16)
    nc.sync.dma_start(wp_sb, aps["wp"])
    ident = cst.tile([P, P], f16)
    nc.sync.dma_start(ident, aps["ident"])
    maskB = cst.tile([P, 1, P], f32)
    nc.sync.dma_start(maskB, aps["maskB"])

    big = ctx.enter_context(tc.tile_pool(name="big", bufs=1))
    qkT = big.tile([P, 4, T], f16)           # sub 0-1: q^T, 2-3: k^T
    v4 = big.tile([P, TQ, HPC, 65], f16)     # v token-major + ones col 64
    yT = big.tile([P, 2, T], f16)            # [0:64]+[64:128] v-dims per e-tile

    xTp = ctx.enter_context(tc.tile_pool(name="xTp", bufs=CFG["xT_bufs"]))
    expSp = ctx.enter_context(tc.tile_pool(name="expSp", bufs=CFG["expS_bufs"]))
    ynp = ctx.enter_context(tc.tile_pool(name="ynp", bufs=3))
    rcpp = ctx.enter_context(tc.tile_pool(name="rcpp", bufs=2))
    zp = ctx.enter_context(tc.tile_pool(name="zp", bufs=3))

    pS = ctx.enter_context(tc.tile_pool(name="pS", bufs=CFG["pS_bufs"], space="PSUM"))
    pY = ctx.enter_context(tc.tile_pool(name="pY", bufs=2, space="PSUM"))
    pG = ctx.enter_context(tc.tile_pool(name="pG", bufs=2, space="PSUM"))

    exp_ctr = [0]

    def emit_exp(pSt, pr, is_diag, wB, expS):
        """exp of a [128, 1024] score pair-tile (blocks 2pr, 2pr+1) into
        expS[:, pr, :]. Full pairs: one instr, engine by CFG ratio. Diag
        pairs: one plain exp over [128 : 512+wB], then one fused staircase
        mask-exp (DVE) overwriting cols [0:128] and [512:640] via a strided
        AP. Junk cols between a short block A and col 512 are exp'd but
        never read."""
        use_act = (exp_ctr[0] % 8) < CFG["act8"]
        exp_ctr[0] += 1
        if not is_diag:
            if use_act:
                nc.scalar.activation(expS[:, pr, :], pSt, FA.Exp)
            else:
                nc.vector.tensor_scalar(
                    expS[:, pr, :].bitcast(i16), pSt, A16, B16, MUL, ADD
                )
            return
        hi = 512 + wB
        if use_act:
            nc.scalar.activation(expS[:, pr, 128:hi], pSt[:, 128:hi], FA.Exp)
        else:
            nc.vector.tensor_scalar(
                expS[:, pr, 128:hi].bitcast(i16), pSt[:, 128:hi], A16, B16,
                MUL, ADD,
            )
        ev = expS[:, pr, :].rearrange("p (a b) -> p a b", b=512)
        pv_ = pSt.rearrange("p (a b) -> p a b", b=512)
        nc.vector.scalar_tensor_tensor(
            ev[:, :, 0:128].bitcast(i16), pv_[:, :, 0:128], A16,
            maskB[:, 0:1, :].to_broadcast((P, 2, 128)), MUL, ADD,
        )

    def phase1_chunks(xT, tg):
        """qkv projection for token group tg, yield per chunk."""
        for jc in range(4):
            pq = pG.tile([P, 512], f32, tag="g", name=f"pq_{tg}_{jc}")
            for eo in range(EO):
                nc.tensor.matmul(
                    pq,
                    lhsT=wqk_sb[:, eo, ts(jc, P)],
                    rhs=xT[:, eo, ts(tg, 512)],
                    start=(eo == 0),
                    stop=(eo == EO - 1),
                )
            nc.scalar.activation(
                qkT[:, jc, ts(tg, 512)], pq, FA.Identity, bias=bqk_sb[:, jc : jc + 1]
            )
            yield
        for j in range(4):
            tq = 4 * tg + j
            pv = pG.tile([P, 512], f32, tag="g", name=f"pv_{tq}")
            for eo in range(EO):
                nc.tensor.matmul(
                    pv[:, :EC],
                    lhsT=xT[:, eo, ts(tq, P)],
                    rhs=wv_sb[:, eo, :],
                    start=(eo == 0),
                    stop=(eo == EO - 1),
                )
            nc.scalar.activation(
                v4[:, tq, :, 0:64],
                pv[:, :EC].rearrange("p (h c) -> p h c", c=64),
                FA.Copy,
            )
            yield

    def load_xT():
        xT = xTp.tile([P, EO, T], f16, tag="x", name="xT_sb")
        for eo in range(0, EO, 2):
            for th in range(4):
                nc.sync.dma_start(
                    xT[:, eo : eo + 2, ts(th, T // 4)],
                    aps["xT"][:, eo : eo + 2, ts(th, T // 4)],
                )
        return xT

    def scores_chunks(qg, expS4):
        """S^T block-pairs + exp for all 4 heads of query group qg.

        Per (head, pair pr): one [128, 1024] psum tile holding blocks
        (2pr, 2pr+1), each a left-packed [128, w] matmul; head pairs on
        disjoint PE row groups run concurrently.
        expS4 = 4 per-head expS tiles [P, 8, 1024] f16."""
        npr = 2 * qg + 2
        for pr in range(npr):
            is_diag = 2 * pr >= 4 * qg
            wB = 512 - 128 * max(2 * pr + 1 - 4 * qg, 0)
            for pair in range(2):
                q_sub, k_sub = pair, 2 + pair
                tiles = []
                for j in range(2):
                    h = 2 * pair + j
                    hp = j * 64
                    pSt = pS.tile([P, 1024], f32, tag="s", name=f"pS_{qg}_{h}_{pr}")
                    tiles.append(pSt)
                    for kk in range(2):
                        kb = 2 * pr + kk
                        jj = max(kb - 4 * qg, 0)
                        w = 512 - 128 * jj
                        nc.tensor.matmul(
                            pSt[:, 512 * kk : 512 * kk + w],
                            lhsT=qkT[hp : hp + 64, k_sub, ts(kb, P)],
                            rhs=qkT[
                                hp : hp + 64, q_sub,
                                qg * 512 + 128 * jj : (qg + 1) * 512,
                            ],
                            start=True,
                            stop=True,
                        )
                for j in range(2):
                    h = 2 * pair + j
                    emit_exp(tiles[j], pr, is_diag, wB, expS4[h])
                yield

    def pv_y(qg, expS4, sgen):
        """y-form PV + den for query group qg, one query chunk at a time.

        Per chunk c: ypc [128, 4h, 65] psum (one bank); for each key block
        kb <= 4qg+c and head h one matmul with rhs = [v | ones] [128, 65] —
        col 64 accumulates the softmax denominator. Then normalize+transpose
        for c while chunk c+1 accumulates (pY double-buffered).

        PSUM start=True lazily zeroes the whole 2KB bank, so only the first
        matmul of the chunk carries it; other heads' first writes land on
        pending-zero bytes and overwrite, then accumulate."""
        for c in range(4):
            ypc = pY.tile([P, HPC, 65], f32, tag="y", name=f"yps_{qg}_{c}")
            last_kb = 4 * qg + c
            for kb in range(last_kb + 1):
                jj = max(kb - 4 * qg, 0)
                off = 512 * (kb % 2) + 128 * (c - jj)
                for h in range(HPC):
                    nc.tensor.matmul(
                        ypc[:, h, :],
                        lhsT=expS4[h][:, kb // 2, off : off + 128],
                        rhs=v4[:, kb, h, :],
                        start=(kb == 0 and h == 0),
                        stop=(kb == last_kb),
                        skip_group_check=True,
                    )
                _pull(sgen)
            normalize_c(qg, c, ypc, sgen)

    def normalize_c(qg, c, ypc, sgen):
        """reciprocal of den col (DVE) -> batched broadcast-multiply (DVE)
        -> transposes (PE) -> yT copy (ScalarE)."""
        rcp = rcpp.tile([P, 4, 1], f32, tag="r", name=f"rcp_{qg}_{c}")
        nc.vector.reciprocal_approx_fast(rcp, ypc[:, :, 64:65])
        yn = ynp.tile([P, HPC, D], f16, tag="n", name=f"yn_{qg}_{c}")
        nc.vector.tensor_tensor(
            yn, ypc[:, :, 0:64], rcp.to_broadcast((P, HPC, D)), MUL
        )
        tpt = pG.tile([P, 2, P], f16, tag="g", name=f"tpt_{qg}_{c}")
        for e in range(2):
            nc.tensor.matmul(
                tpt[:, e, :],
                lhsT=yn[:, 2 * e : 2 * e + 2, :],
                rhs=ident,
                start=True,
                stop=True,
                is_transpose=True,
            )
        nc.scalar.activation(
            yT[:, :, qg * 512 + 128 * c : qg * 512 + 128 * (c + 1)], tpt,
            FA.Copy,
        )
        _pull(sgen)

    def proj_z(qg, sgen):
        """output projection for the 4 token chunks of query group qg;
        PSUM -> SBUF f16 copy on DVE, then DMA (bias added on host)."""
        for tq in range(4 * qg, 4 * qg + 4):
            pz = pG.tile([P, 512], f32, tag="g", name=f"pz_{tq}")
            for e in range(2):
                nc.tensor.matmul(
                    pz,
                    lhsT=yT[:, e, ts(tq, P)],
                    rhs=wp_sb[:, e, :],
                    start=(e == 0),
                    stop=(e == 1),
                )
            zt = zp.tile([P, E], f16, tag="z", name=f"zt_{tq}")
            if tq % 2 == 0:
                nc.vector.tensor_copy(zt, pz)
            else:
                nc.scalar.activation(zt, pz, FA.Copy)
            nc.sync.dma_start(z[ts(tq, P), :], zt)
            _pull(sgen)

    class Pacer:
        """Spreads `items` generator steps evenly over `slots` pull calls."""

        def __init__(self, gen, items, slots):
            self.gen = gen
            self.rate = items / max(slots, 1)
            self.acc = 0.0

        def pull(self):
            if self.gen is None:
                return
            self.acc += self.rate
            while self.acc >= 1.0:
                self.acc -= 1.0
                try:
                    next(self.gen)
                except StopIteration:
                    self.gen = None
                    return

        def drain(self):
            if self.gen is not None:
                for _ in self.gen:
                    pass
                self.gen = None

    def _pull(gen):
        if gen is not None:
            gen.pull()

    def _drain(gen):
        if gen is None:
            return
        if isinstance(gen, Pacer):
            gen.drain()
        else:
            for _ in gen:
                pass

    def phase1(xT, tg):
        _drain(phase1_chunks(xT, tg))

    def new_exp4(qg):
        return [
            expSp.tile([P, 8, 1024], f16, tag="e", name=f"exp{j}_{qg}")
            for j in range(4)
        ]

    nc.vector.memset(v4[:, :, :, 64], 1.0)
    xT = load_xT()
    phase1(xT, 0)
    exp4 = new_exp4(0)
    for _ in scores_chunks(0, exp4):
        pass
    for r in range(reps):
        last = r == reps - 1
        xTn = None
        for qg in range(NQG):
            # dense PE work for qg, interleaved with scores+exp of qg+1;
            # pulls paced so the interleaved stream spans the whole group
            slots = 16 * qg + 18
            if qg == NQG - 2 and not last:
                xTn = load_xT()  # prefetch next rep's x well before use
            if qg < NQG - 1:
                phase1(xT, qg + 1)
                exp4n = new_exp4(qg + 1)
                items = 2 * (2 * (qg + 1) + 2)
                sgen = Pacer(scores_chunks(qg + 1, exp4n), items, slots)
            elif not last:
                exp4n = new_exp4(0)
                sgen = Pacer(
                    _chain(phase1_chunks(xTn, 0), scores_chunks(0, exp4n)),
                    12, slots,
                )
            else:
                exp4n, sgen = None, None
            pv_y(qg, exp4, sgen)
            if qg > 0:
                proj_z(qg - 1, sgen)
            _drain(sgen)
            exp4 = exp4n
            if qg == NQG - 1 and not last:
                xT = xTn
        proj_z(NQG - 1, None)


def build(reps=1):
    nc = bacc.Bacc("TRN2", target_bir_lowering=False, debug=False)
    aps = {
        "xT": nc.dram_tensor("xT", [P, EO, T], f16, kind="ExternalInput").ap(),
        "wqk": nc.dram_tensor("wqk", [P, EO, 2 * EC], f16, kind="ExternalInput").ap(),
        "bqk": nc.dram_tensor("bqk", [P, 4], f32, kind="ExternalInput").ap(),
        "wv": nc.dram_tensor("wv", [P, EO, EC], f16, kind="ExternalInput").ap(),
        "wp": nc.dram_tensor("wp", [P, 2, E], f16, kind="ExternalInput").ap(),
        "ident": nc.dram_tensor("ident", [P, P], f16, kind="ExternalInput").ap(),
        "maskB": nc.dram_tensor("maskB", [P, P], f32, kind="ExternalInput").ap(),
        "z": nc.dram_tensor("z", [T, E], f16, kind="ExternalOutput").ap(),
    }
    with tile.TileContext(nc) as tc, ExitStack() as ctx:
        _emit(tc, ctx, aps, reps=reps)
    nc.compile()
    return nc


def make_in_maps(x, c_attn_w, c_attn_b, c_proj_w, c_proj_b):
    x = np.asarray(x, np.float32)
    W = np.asarray(c_attn_w, np.float32)
    bW = np.asarray(c_attn_b, np.float32)

    ident = np.eye(P, dtype=np.float16)
    # maskB[p, c] = B16 if query col c >= key row p else -1e5 (staircase)
    pp = np.arange(P)[:, None]
    cc = np.arange(P)[None, :]
    maskB = np.where(cc >= pp, B16, MASKNEG).astype(np.float32)
    in_maps = []
    for c in range(NCORES):
        b, hg = c // 2, c % 2
        qs = slice(hg * EC, (hg + 1) * EC)
        ks = slice(E + hg * EC, E + (hg + 1) * EC)
        vs = slice(2 * E + hg * EC, 2 * E + (hg + 1) * EC)
        wqk = np.concatenate([W[:, qs] * SCALE, W[:, ks]], axis=1)  # [512, 512]
        bqk = np.concatenate([bW[qs] * SCALE, bW[ks]])              # [512]
        xT = np.ascontiguousarray(
            x[b].T.reshape(EO, P, T).transpose(1, 0, 2)
        ).astype(np.float16)
        Wp_core = np.asarray(c_proj_w, np.float32)[hg * EC : (hg + 1) * EC, :]
        in_maps.append({
            "xT": xT,
            "wqk": np.ascontiguousarray(
                wqk.reshape(EO, P, 2 * EC).transpose(1, 0, 2)
            ).astype(np.float16),
            "bqk": np.ascontiguousarray(bqk.reshape(4, P).T),
            "wv": np.ascontiguousarray(
                W[:, vs].reshape(EO, P, EC).transpose(1, 0, 2)
            ).astype(np.float16),
            "wp": np.ascontiguousarray(
                Wp_core.reshape(2, P, E).transpose(1, 0, 2)
            ).astype(np.float16),
            "ident": ident,
            "maskB": maskB,
        })
    return in_maps


_NC_CACHE = {}


def kernel(x, c_attn_w, c_attn_b, c_proj_w, c_proj_b):
    if "nc" not in _NC_CACHE:
        _NC_CACHE["nc"] = build()
    nc = _NC_CACHE["nc"]
    in_maps = make_in_maps(x, c_attn_w, c_attn_b, c_proj_w, c_proj_b)
    res = run_bass_kernel_spmd(nc, in_maps, core_ids=list(range(NCORES)))
    bW = np.asarray(c_attn_b, np.float32)
    Wp = np.asarray(c_proj_w, np.float32)
    bias = np.asarray(c_proj_b, np.float32) + bW[2 * E :] @ Wp  # [512]
    out = np.empty((B, T, E), np.float32)
    for b in range(B):
        out[b] = (
            res.results[2 * b]["z"].astype(np.float32)
            + res.results[2 * b + 1]["z"].astype(np.float32)
            + bias[None, :]
        )
    return out
